# revision 14
# baseline (speedup 1.0000x reference)
"""Trainium2 Bass kernel for nn_CrossAttnModule (B=8,N1=N2=4096,C=512,P=256,H=8,MLP=2048).

Sharding: data-parallel over B across the 8 NeuronCores (one batch element per
core); weights replicated. All matmul inputs in bf16 (inputs/weights converted
on host); PSUM accumulation in fp32. Structured to minimize activation-table
switches (one func-set for stages A-D, one switch to gelu for the FFN, one
switch back for the final norms), drain PSUM via the Activation engine where
the Vector engine is the bottleneck, and keep all intermediates in SBUF (the
attention-output permute and the q/y transposes run on the DMA XBAR instead of
PE/DVE).
"""
import sys

for _p in ("/opt/trn_rl_repo", "/opt/trn_rl_repo/concourse"):
    if _p not in sys.path:
        sys.path.insert(0, _p)

import numpy as np
import ml_dtypes

B, N1, N2, C, P, H, MLP = 8, 4096, 4096, 512, 256, 8, 2048
HD = C // H  # 64

_CACHE = {}


def _build(temp_vals, use_bias, use_pos):
    import concourse.bass as bass
    import concourse.bacc as bacc
    import concourse.mybir as mybir
    import concourse.tile as tile

    dt = mybir.dt
    AFT = mybir.ActivationFunctionType
    ALU = mybir.AluOpType
    f32, bf16 = dt.float32, dt.bfloat16

    nc = bacc.Bacc("TRN2", target_bir_lowering=False, debug=False, num_devices=8)

    # ---- external I/O (per core), bf16 unless noted ----
    x1T = nc.dram_tensor("x1T", [C, N1], bf16, kind="ExternalInput")
    x2T = nc.dram_tensor("x2T", [C, N2], bf16, kind="ExternalInput")
    Wq = nc.dram_tensor("Wq", [C, C], bf16, kind="ExternalInput")
    Wkv = nc.dram_tensor("Wkv", [C, 2 * C], bf16, kind="ExternalInput")
    Wp = nc.dram_tensor("Wp", [N2, P], bf16, kind="ExternalInput")
    W1 = nc.dram_tensor("W1", [C, MLP], bf16, kind="ExternalInput")
    W2 = nc.dram_tensor("W2", [MLP, C], bf16, kind="ExternalInput")
    if use_bias:
        bq_d = nc.dram_tensor("bq", [C], f32, kind="ExternalInput")
        bkv_d = nc.dram_tensor("bkv2", [2 * C], bf16, kind="ExternalInput")
        bp_d = nc.dram_tensor("bp", [P], f32, kind="ExternalInput")
        bp2_d = nc.dram_tensor("bp2", [P], bf16, kind="ExternalInput")
        b1_d = nc.dram_tensor("b1", [MLP], f32, kind="ExternalInput")
        b2_d = nc.dram_tensor("b2", [C], bf16, kind="ExternalInput")
    if use_pos:
        pqT_d = nc.dram_tensor("pqT", [C, N1], bf16, kind="ExternalInput")
        pkN_d = nc.dram_tensor("pkN", [N2, C], bf16, kind="ExternalInput")
    out_d = nc.dram_tensor("out", [N1, C], bf16, kind="ExternalOutput")

    MAGIC = 0x5F3759DF

    with tile.TileContext(nc) as tc:
        def dve_rsqrt(dst, s_ap, pool, tag, w):
            """dst = 1/sqrt(s) entirely on DVE (bit hack + 1 Newton step),
            batched over a [128, w] block so the serial 6-op chain is paid
            once, not per tile. Keeps Ln/Exp off the scalar engine so the
            whole kernel needs only two activation-table loads."""
            ii = pool.tile([128, w], dt.int32, tag=f"{tag}i", bufs=2, name=f"rs_{tag}i")
            nc.vector.tensor_scalar(ii, s_ap.bitcast(dt.int32), 1, None, ALU.logical_shift_right)
            r0 = pool.tile([128, w], f32, tag=f"{tag}r0", bufs=2, name=f"rs_{tag}r0")
            nc.vector.tensor_scalar(r0.bitcast(dt.int32), ii, MAGIC, -1, ALU.subtract, ALU.mult)
            a = pool.tile([128, w], f32, tag=f"{tag}a", bufs=2, name=f"rs_{tag}a")
            nc.vector.tensor_mul(a, r0, r0)
            nc.vector.tensor_mul(a, a, s_ap)
            nc.vector.tensor_scalar(a, a, -0.5, 1.5, ALU.mult, ALU.add)
            r1 = pool.tile([128, w], f32, tag=f"{tag}r1", bufs=2, name=f"rs_{tag}r1")
            nc.vector.tensor_mul(r1, a, r0)
            nc.vector.tensor_mul(a, r1, r1)
            nc.vector.tensor_mul(a, a, s_ap)
            nc.vector.tensor_scalar(a, a, -0.5, 1.5, ALU.mult, ALU.add)
            nc.vector.tensor_mul(dst, a, r1)

        glob = tc.alloc_tile_pool(name="glob", bufs=1)
        kp_sb = [glob.tile([128, 512], bf16, tag=f"kp{j}", name=f"kp_sb{j}") for j in range(2)]
        vpe = [glob.tile([128, 8, 65], bf16, tag=f"vpe{pc}", name=f"vpe{pc}") for pc in range(2)]
        drec = glob.tile([128, 512], bf16, tag="drec")
        ss1 = glob.tile([128, 32], f32, tag="ss1")
        rn1 = glob.tile([128, 32], f32, tag="rn1")
        ss2 = glob.tile([128, 32], f32, tag="ss2")
        rn2 = glob.tile([128, 32], f32, tag="rn2")
        if use_bias:
            ones1 = glob.tile([1, 128], f32, tag="ones1f")
            nc.vector.memset(ones1, 1.0)
            ones1b = glob.tile([1, 128], bf16, tag="ones1b")
            nc.vector.tensor_copy(ones1b, ones1)
            bq_sb = glob.tile([128, 4], f32, tag="bq")  # [p, co]: bq[co*128+p]
            nc.sync.dma_start(out=bq_sb, in_=bass.AP(tensor=bq_d.ap().tensor, offset=0, ap=[[1, 128], [128, 4]]))
            bkvr = glob.tile([1, 1024], bf16, tag="bkvr")
            nc.sync.dma_start(out=bkvr, in_=bkv_d.ap().unsqueeze(0))
            bp_sb = glob.tile([128, 2], f32, tag="bp")
            nc.sync.dma_start(out=bp_sb, in_=bass.AP(tensor=bp_d.ap().tensor, offset=0, ap=[[1, 128], [128, 2]]))
            bp2r = glob.tile([1, 512], bf16, tag="bp2r")
            nc.sync.dma_start(out=bp2r[:, 0:256], in_=bp2_d.ap().unsqueeze(0))
            nc.sync.dma_start(out=bp2r[:, 256:512], in_=bp2_d.ap().unsqueeze(0))
            b1_sb = glob.tile([128, 16], f32, tag="b1")
            nc.sync.dma_start(out=b1_sb, in_=bass.AP(tensor=b1_d.ap().tensor, offset=0, ap=[[1, 128], [128, 16]]))
            b2r = glob.tile([1, 512], bf16, tag="b2r")
            nc.sync.dma_start(out=b2r, in_=b2_d.ap().unsqueeze(0))

        # persistent cross-phase big tensors (stacked so releases are LIFO)
        ffn_pool = tc.alloc_tile_pool(name="ffn_pool", bufs=1)
        w14 = ffn_pool.tile([128, 4, MLP], bf16, tag="w14")
        w2t = ffn_pool.tile([128, 16, C], bf16, tag="w2t")
        ytin = ffn_pool.tile([128, 4, N1], bf16, tag="ytin")   # [p, cc, tok]: y^T
        ytk = ffn_pool.tile([128, 32, 512], bf16, tag="ytk")   # [p, i, c]: y rows (then z2 in-place)
        qn_pool = tc.alloc_tile_pool(name="qn_pool", bufs=1)
        qn_sb = qn_pool.tile([128, 32, 512], bf16, tag="qn")  # qn_sb[p,i,:] = q[i*128+p,:]
        qt_pool = tc.alloc_tile_pool(name="qt_pool", bufs=1)
        qt_res = [qt_pool.tile([128, N1], bf16, tag=f"qtr{co}", name=f"qt_res{co}") for co in range(4)]
        # attention-output bounce buffer: rows d*64+m for d in 0..63, den at 4096+m
        xperm2 = nc.dram_tensor("xperm2", [65 * 64, 512], bf16)

        # ================= stage A: q^T (+ qnat via XBAR transpose) =================
        with tc.tile_pool(name="ab_sb", bufs=1) as ab:
            wq4 = ab.tile([128, 4, C], bf16, tag="wq4")
            nc.sync.dma_start(out=wq4, in_=Wq.ap().rearrange("(cc p) n -> p cc n", p=128))
            aps_pool = tc.alloc_tile_pool(name="a_ps", bufs=1, space="PSUM")
            for nq in range(8):
                x1t4 = ab.tile([128, 4, 512], bf16, tag="x1t4", bufs=2)
                nc.sync.dma_start(out=x1t4, in_=x1T.ap().rearrange("(cc p) n -> p cc n", p=128)[:, :, nq * 512:(nq + 1) * 512])
                if use_pos:
                    pqt_t = ab.tile([128, 4, 512], bf16, tag="pqt")
                    nc.sync.dma_start(out=pqt_t, in_=pqT_d.ap().rearrange("(a p) n -> p a n", p=128)[:, :, nq * 512:(nq + 1) * 512])
                for co in range(4):
                    ps = aps_pool.tile([128, 512], f32, tag="qt_ps", bufs=2)
                    for cc in range(4):
                        nc.tensor.matmul(ps, wq4[:, cc, co * 128:(co + 1) * 128], x1t4[:, cc, :],
                                         start=(cc == 0), stop=(cc == 3))
                    dst = qt_res[co][:, nq * 512:(nq + 1) * 512]
                    if use_bias:
                        nc.scalar.activation(dst, ps, AFT.Identity, bias=bq_sb[:, co:co + 1])
                        if use_pos:
                            nc.vector.tensor_add(dst, dst, pqt_t[:, co, :])
                    elif use_pos:
                        nc.vector.tensor_add(dst, ps, pqt_t[:, co, :])
                    else:
                        nc.scalar.activation(dst, ps, AFT.Identity)
            aps_pool.release()
            # qnat: qn_sb[p, i, co*128+c2] = qt_res[co][c2, i*128+p] via DMA XBAR transpose
            for co in range(4):
                nc.sync.dma_start(out=qn_sb[:, :, co * 128:(co + 1) * 128], in_=qt_res[co], transpose=True)

            # ================= stage B: k, v, kp, vpT =================
            bps_pool = tc.alloc_tile_pool(name="b_ps", bufs=1, space="PSUM")
            wkv4 = ab.tile([128, 4, 2 * C], bf16, tag="wkv4")
            nc.sync.dma_start(out=wkv4, in_=Wkv.ap().rearrange("(cc p) n -> p cc n", p=128))
            kp_ps = [bps_pool.tile([128, 256], f32, tag=f"kp_ps{j}", name=f"kp_ps{j}") for j in range(4)]
            vp_ps = [bps_pool.tile([128, 512], f32, tag=f"vp_ps{pc}", name=f"vp_ps{pc}") for pc in range(2)]
            x2t4 = wp4 = None
            for n2c in range(32):
                blk, sl = n2c // 4, n2c % 4
                if sl == 0:
                    x2t4 = ab.tile([128, 4, 512], bf16, tag="x2t4", bufs=2)
                    nc.sync.dma_start(out=x2t4, in_=x2T.ap().rearrange("(cc p) n -> p cc n", p=128)[:, :, blk * 512:(blk + 1) * 512])
                    wp4 = ab.tile([128, 4, 256], bf16, tag="wp4", bufs=2)
                    nc.sync.dma_start(out=wp4, in_=Wp.ap().rearrange("(a p) n -> p a n", p=128)[:, blk * 4:blk * 4 + 4, :])
                wp_in = wp4[:, sl, :]
                kps = bps_pool.tile([128, 512], f32, tag="k_ps", bufs=1)
                vps = bps_pool.tile([128, 512], f32, tag="v_ps", bufs=1)
                for cc in range(4):
                    nc.tensor.matmul(kps, x2t4[:, cc, sl * 128:(sl + 1) * 128], wkv4[:, cc, 0:512],
                                     start=(cc == 0), stop=(cc == 3 and not use_bias))
                if use_bias:
                    nc.tensor.matmul(kps, ones1b, bkvr[:, 0:512], start=False, stop=True, skip_group_check=True)
                for cc in range(4):
                    nc.tensor.matmul(vps, x2t4[:, cc, sl * 128:(sl + 1) * 128], wkv4[:, cc, 512:1024],
                                     start=(cc == 0), stop=(cc == 3 and not use_bias))
                if use_bias:
                    nc.tensor.matmul(vps, ones1b, bkvr[:, 512:1024], start=False, stop=True, skip_group_check=True)
                k_sb = ab.tile([128, 512], bf16, tag="k_sb", bufs=3)
                v_sb = ab.tile([128, 512], bf16, tag="v_sb", bufs=3)
                nc.scalar.activation(k_sb, kps, AFT.Identity)
                if use_pos:
                    pk_t = ab.tile([128, 512], bf16, tag="pkn", bufs=2)
                    nc.sync.dma_start(out=pk_t, in_=pkN_d.ap()[n2c * 128:(n2c + 1) * 128, :])
                    nc.vector.tensor_add(k_sb, k_sb, pk_t)
                nc.vector.tensor_copy(v_sb, vps)
                for hp in range(4):
                    nc.tensor.matmul(kp_ps[hp], k_sb[:, hp * 128:(hp + 1) * 128], wp_in,
                                     start=(n2c == 0), stop=(n2c == 31 and not use_bias))
                for pc in range(2):
                    nc.tensor.matmul(vp_ps[pc], wp_in[:, pc * 128:(pc + 1) * 128], v_sb,
                                     start=(n2c == 0), stop=(n2c == 31))
            if use_bias:
                for hp in range(4):
                    nc.tensor.matmul(kp_ps[hp], ones1b, bp2r[:, 0:256], start=False, stop=True, skip_group_check=True)
            for hp in range(4):
                nc.vector.tensor_copy(kp_sb[hp // 2][:, (hp % 2) * 256:(hp % 2) * 256 + 256], kp_ps[hp])
            for pc in range(2):
                vv = vpe[pc]
                if use_bias:
                    nc.scalar.activation(vv[:, :, 0:64], vp_ps[pc].rearrange("p (h e) -> p h e", e=64),
                                         AFT.Identity, bias=bp_sb[:, pc:pc + 1])
                else:
                    nc.scalar.activation(vv[:, :, 0:64], vp_ps[pc].rearrange("p (h e) -> p h e", e=64),
                                         AFT.Identity)
                of = glob.tile([128, 8], bf16, tag="onesf")
                nc.vector.memset(of, 1.0)
                nc.vector.tensor_copy(vv[:, :, 64:65], of.unsqueeze(2))
            bps_pool.release()

        # FFN weights load during the (activation-bound) attention phase
        nc.sync.dma_start(out=w14, in_=W1.ap().rearrange("(cc p) n -> p cc n", p=128))
        nc.sync.dma_start(out=w2t, in_=W2.ap().rearrange("(m p) n -> p m n", p=128))

        # ================= stage C: attention =================
        with tc.tile_pool(name="c_sb", bufs=1) as cp, \
             tc.tile_pool(name="c_ps", bufs=1, space="PSUM") as cps:
            for nq in range(8):
                xsm = cp.tile([65, 8, 512], bf16, tag="xsm", bufs=2)
                for t in range(4):
                    for hh in range(2):
                        h = 2 * t + hh
                        aps = cps.tile([128, 2, 512], f32, tag="attn_ps", bufs=3)
                        for pc in range(2):
                            nc.tensor.matmul(aps[:, pc, :],
                                             kp_sb[t // 2][hh * 64:(hh + 1) * 64, (t % 2) * 256 + pc * 128:(t % 2) * 256 + (pc + 1) * 128],
                                             qt_res[t][hh * 64:(hh + 1) * 64, nq * 512:(nq + 1) * 512],
                                             start=True, stop=True)
                        ebuf = cp.tile([128, 2, 512], bf16, tag="ebuf", bufs=4)
                        nc.scalar.activation(ebuf, aps, AFT.Exp, scale=float(temp_vals[h]))
                        xps = cps.tile([65, 512], f32, tag="x_ps", bufs=2)
                        nc.tensor.matmul(xps, vpe[0][:, h, :], ebuf[:, 0, :], start=True, stop=False)
                        nc.tensor.matmul(xps, vpe[1][:, h, :], ebuf[:, 1, :], start=False, stop=True)
                        nc.vector.tensor_copy(xsm[:, h, :], xps)
                nc.sync.dma_start(
                    out=bass.AP(tensor=xperm2.ap().tensor, offset=nq * 512,
                                ap=[[64 * 512, 65], [8 * 512, 8], [1, 512]]),
                    in_=xsm)
        qt_pool.release()

        # ================= stage D: permute + add&norm + y^T =================
        with tc.tile_pool(name="d_sb", bufs=1) as dp:
            denb = dp.tile([128, 512], bf16, tag="denb")
            for dl in range(2):
                nc.sync.dma_start(out=denb[dl * 64:(dl + 1) * 64, :], in_=xperm2.ap()[4096:4160, :])
            with nc.allow_low_precision(reason="softmax denominators are O(100+); bf16 reciprocal is plenty for 2e-2 tol"):
                nc.vector.reciprocal(drec, denb)
            for g in range(4):
                zts = []
                xt8 = dp.tile([128, 8, 512], bf16, tag="xt8", bufs=2)
                nc.sync.dma_start(out=xt8, in_=bass.AP(tensor=xperm2.ap().tensor, offset=g * 8 * 128 * 512,
                                                       ap=[[512, 128], [128 * 512, 8], [1, 512]]))
                for i in range(g * 8, g * 8 + 8):
                    zt = dp.tile([128, 512], bf16, tag=f"zt{i % 8}", name=f"zt{i % 8}", bufs=2)
                    nc.vector.tensor_mul(zt, xt8[:, i % 8, :], drec)
                    nc.vector.tensor_add(zt, zt, qn_sb[:, i, :])
                    sq = dp.tile([128, 512], bf16, tag="sq", bufs=3)
                    nc.scalar.activation(sq, zt, AFT.Square, accum_out=ss1[:, i:i + 1])
                    zts.append(zt)
                dve_rsqrt(rn1[:, g * 8:g * 8 + 8], ss1[:, g * 8:g * 8 + 8], dp, "d", 8)
                for i in range(g * 8, g * 8 + 8):
                    nc.vector.tensor_scalar_mul(ytk[:, i, :], zts[i - g * 8], rn1[:, i:i + 1])
                    nc.sync.dma_start(out=ytin[:, :, i * 128:(i + 1) * 128], in_=ytk[:, i, :], transpose=True)
        qn_pool.release()

        # ================= stage E: FFN + final norm =================
        with tc.tile_pool(name="e_sb", bufs=1) as ep, \
             tc.tile_pool(name="e_ps", bufs=1, space="PSUM") as eps:
            for nq in range(8):
                h1t = []
                for m in range(16):
                    ps = eps.tile([128, 512], f32, tag="h1_ps", bufs=4)
                    for cc in range(4):
                        nc.tensor.matmul(ps, w14[:, cc, m * 128:(m + 1) * 128],
                                         ytin[:, cc, nq * 512:(nq + 1) * 512],
                                         start=(cc == 0), stop=(cc == 3))
                    ht = ep.tile([128, 512], bf16, tag=f"h1t{m}", name=f"h1t{m}", bufs=2)
                    if use_bias:
                        nc.scalar.activation(ht, ps, AFT.Gelu, bias=b1_sb[:, m:m + 1])
                    else:
                        nc.scalar.activation(ht, ps, AFT.Gelu)
                    h1t.append(ht)
                for sub in range(4):
                    i = nq * 4 + sub
                    ps = eps.tile([128, 512], f32, tag="h2_ps", bufs=2)
                    for m in range(16):
                        nc.tensor.matmul(ps, h1t[m][:, sub * 128:(sub + 1) * 128], w2t[:, m, :],
                                         start=(m == 0), stop=(m == 15 and not use_bias))
                    if use_bias:
                        nc.tensor.matmul(ps, ones1b, b2r, start=False, stop=True, skip_group_check=True)
                    z2 = ytk[:, i, :]
                    nc.vector.tensor_add(z2, ps, ytk[:, i, :])
                    sq2 = ep.tile([128, 512], bf16, tag="sq2", bufs=2)
                    nc.scalar.activation(sq2, z2, AFT.Square, accum_out=ss2[:, i:i + 1])
                # flush final norms for this nq block (DVE-only rsqrt; no
                # act-table switch) so the output tail overlaps the FFN
                g4 = nq * 4
                dve_rsqrt(rn2[:, g4:g4 + 4], ss2[:, g4:g4 + 4], ep, "e", 4)
                ot4 = ep.tile([128, 4, 512], bf16, tag="ot4", bufs=2)
                for i in range(g4, g4 + 4):
                    nc.vector.tensor_scalar_mul(ot4[:, i - g4, :], ytk[:, i, :], rn2[:, i:i + 1])
                nc.sync.dma_start(out=bass.AP(tensor=out_d.ap().tensor, offset=nq * 4 * 128 * 512,
                                              ap=[[512, 128], [128 * 512, 4], [1, 512]]),
                                  in_=ot4)
        ffn_pool.release()
        glob.release()
    nc.compile()
    return nc


def kernel(**inputs):
    from concourse.bass_utils import run_bass_kernel_spmd

    bf = ml_dtypes.bfloat16
    x1 = np.asarray(inputs["x1"], np.float32)
    x2 = np.asarray(inputs["x2"], np.float32)
    temp = np.asarray(inputs["temperature"], np.float32).reshape(H)
    biases = [np.asarray(inputs[k], np.float32) for k in ("bq", "bkv", "bp", "b1", "b2")]
    use_bias = any(np.any(b) for b in biases)
    pos_q = np.asarray(inputs["pos_q"], np.float32).reshape(N1, C)
    pos_k = np.asarray(inputs["pos_k"], np.float32).reshape(N2, C)
    use_pos = bool(np.any(pos_q) or np.any(pos_k))

    key = (tuple(np.round(temp, 7).tolist()), use_bias, use_pos)
    if key not in _CACHE:
        _CACHE[key] = _build(temp, use_bias, use_pos)
    nc = _CACHE[key]

    shared = {
        "Wq": np.ascontiguousarray(inputs["Wq"]).astype(bf),
        "Wkv": np.ascontiguousarray(inputs["Wkv"]).astype(bf),
        "Wp": np.ascontiguousarray(inputs["Wp"]).astype(bf),
        "W1": np.ascontiguousarray(inputs["W1"]).astype(bf),
        "W2": np.ascontiguousarray(inputs["W2"]).astype(bf),
    }
    if use_bias:
        shared.update(bq=biases[0], bkv2=biases[1].astype(bf), bp=biases[2],
                      bp2=biases[2].astype(bf), b1=biases[3], b2=biases[4].astype(bf))
    if use_pos:
        shared.update(pqT=np.ascontiguousarray(pos_q.T).astype(bf), pkN=pos_k.astype(bf))
    in_maps = []
    for b in range(B):
        m = dict(shared)
        m["x1T"] = np.ascontiguousarray(x1[b].T).astype(bf)
        m["x2T"] = np.ascontiguousarray(x2[b].T).astype(bf)
        in_maps.append(m)
    res = run_bass_kernel_spmd(nc, in_maps, core_ids=list(range(B)))
    return np.stack([np.asarray(res.results[b]["out"]).astype(np.float32) for b in range(B)])


# revision 15
# speedup vs baseline: 1.0041x; 1.0041x over previous
"""Trainium2 Bass kernel for nn_CrossAttnModule (B=8,N1=N2=4096,C=512,P=256,H=8,MLP=2048).

Sharding: data-parallel over B across the 8 NeuronCores (one batch element per
core); weights replicated. All matmul inputs in bf16 (inputs/weights converted
on host); PSUM accumulation in fp32. Structured to minimize activation-table
switches (one func-set for stages A-D, one switch to gelu for the FFN, one
switch back for the final norms), drain PSUM via the Activation engine where
the Vector engine is the bottleneck, and keep all intermediates in SBUF (the
attention-output permute and the q/y transposes run on the DMA XBAR instead of
PE/DVE).
"""
import sys

for _p in ("/opt/trn_rl_repo", "/opt/trn_rl_repo/concourse"):
    if _p not in sys.path:
        sys.path.insert(0, _p)

import numpy as np
import ml_dtypes

B, N1, N2, C, P, H, MLP = 8, 4096, 4096, 512, 256, 8, 2048
HD = C // H  # 64

_CACHE = {}


def _build(temp_vals, use_bias, use_pos):
    import concourse.bass as bass
    import concourse.bacc as bacc
    import concourse.mybir as mybir
    import concourse.tile as tile

    dt = mybir.dt
    AFT = mybir.ActivationFunctionType
    ALU = mybir.AluOpType
    f32, bf16 = dt.float32, dt.bfloat16

    nc = bacc.Bacc("TRN2", target_bir_lowering=False, debug=False, num_devices=8)

    # ---- external I/O (per core), bf16 unless noted ----
    x1T = nc.dram_tensor("x1T", [C, N1], bf16, kind="ExternalInput")
    x2T = nc.dram_tensor("x2T", [C, N2], bf16, kind="ExternalInput")
    Wq = nc.dram_tensor("Wq", [C, C], bf16, kind="ExternalInput")
    Wkv = nc.dram_tensor("Wkv", [C, 2 * C], bf16, kind="ExternalInput")
    Wp = nc.dram_tensor("Wp", [N2, P], bf16, kind="ExternalInput")
    W1 = nc.dram_tensor("W1", [C, MLP], bf16, kind="ExternalInput")
    W2 = nc.dram_tensor("W2", [MLP, C], bf16, kind="ExternalInput")
    if use_bias:
        bq_d = nc.dram_tensor("bq", [C], f32, kind="ExternalInput")
        bkv_d = nc.dram_tensor("bkv2", [2 * C], bf16, kind="ExternalInput")
        bp_d = nc.dram_tensor("bp", [P], f32, kind="ExternalInput")
        bp2_d = nc.dram_tensor("bp2", [P], bf16, kind="ExternalInput")
        b1_d = nc.dram_tensor("b1", [MLP], f32, kind="ExternalInput")
        b2_d = nc.dram_tensor("b2", [C], bf16, kind="ExternalInput")
    if use_pos:
        pqT_d = nc.dram_tensor("pqT", [C, N1], bf16, kind="ExternalInput")
        pkN_d = nc.dram_tensor("pkN", [N2, C], bf16, kind="ExternalInput")
    out_d = nc.dram_tensor("out", [N1, C], bf16, kind="ExternalOutput")

    MAGIC = 0x5F3759DF

    with tile.TileContext(nc) as tc:
        def dve_rsqrt(dst, s_ap, pool, tag, w):
            """dst = 1/sqrt(s) entirely on DVE (bit hack + 1 Newton step),
            batched over a [128, w] block so the serial 6-op chain is paid
            once, not per tile. Keeps Ln/Exp off the scalar engine so the
            whole kernel needs only two activation-table loads."""
            ii = pool.tile([128, w], dt.int32, tag=f"{tag}i", bufs=2, name=f"rs_{tag}i")
            nc.vector.tensor_scalar(ii, s_ap.bitcast(dt.int32), 1, None, ALU.logical_shift_right)
            r0 = pool.tile([128, w], f32, tag=f"{tag}r0", bufs=2, name=f"rs_{tag}r0")
            nc.vector.tensor_scalar(r0.bitcast(dt.int32), ii, MAGIC, -1, ALU.subtract, ALU.mult)
            a = pool.tile([128, w], f32, tag=f"{tag}a", bufs=2, name=f"rs_{tag}a")
            nc.vector.tensor_mul(a, r0, r0)
            nc.vector.tensor_mul(a, a, s_ap)
            nc.vector.tensor_scalar(a, a, -0.5, 1.5, ALU.mult, ALU.add)
            r1 = pool.tile([128, w], f32, tag=f"{tag}r1", bufs=2, name=f"rs_{tag}r1")
            nc.vector.tensor_mul(r1, a, r0)
            nc.vector.tensor_mul(a, r1, r1)
            nc.vector.tensor_mul(a, a, s_ap)
            nc.vector.tensor_scalar(a, a, -0.5, 1.5, ALU.mult, ALU.add)
            nc.vector.tensor_mul(dst, a, r1)

        glob = tc.alloc_tile_pool(name="glob", bufs=1)
        kp_sb = [glob.tile([128, 512], bf16, tag=f"kp{j}", name=f"kp_sb{j}") for j in range(2)]
        vpe = [glob.tile([128, 8, 65], bf16, tag=f"vpe{pc}", name=f"vpe{pc}") for pc in range(2)]
        drec = glob.tile([128, 512], bf16, tag="drec")
        ss1 = glob.tile([128, 32], f32, tag="ss1")
        rn1 = glob.tile([128, 32], f32, tag="rn1")
        ss2 = glob.tile([128, 32], f32, tag="ss2")
        rn2 = glob.tile([128, 32], f32, tag="rn2")
        if use_bias:
            ones1 = glob.tile([1, 128], f32, tag="ones1f")
            nc.vector.memset(ones1, 1.0)
            ones1b = glob.tile([1, 128], bf16, tag="ones1b")
            nc.vector.tensor_copy(ones1b, ones1)
            bq_sb = glob.tile([128, 4], f32, tag="bq")  # [p, co]: bq[co*128+p]
            nc.sync.dma_start(out=bq_sb, in_=bass.AP(tensor=bq_d.ap().tensor, offset=0, ap=[[1, 128], [128, 4]]))
            bkvr = glob.tile([1, 1024], bf16, tag="bkvr")
            nc.sync.dma_start(out=bkvr, in_=bkv_d.ap().unsqueeze(0))
            bp_sb = glob.tile([128, 2], f32, tag="bp")
            nc.sync.dma_start(out=bp_sb, in_=bass.AP(tensor=bp_d.ap().tensor, offset=0, ap=[[1, 128], [128, 2]]))
            bp2r = glob.tile([1, 512], bf16, tag="bp2r")
            nc.sync.dma_start(out=bp2r[:, 0:256], in_=bp2_d.ap().unsqueeze(0))
            nc.sync.dma_start(out=bp2r[:, 256:512], in_=bp2_d.ap().unsqueeze(0))
            b1_sb = glob.tile([128, 16], f32, tag="b1")
            nc.sync.dma_start(out=b1_sb, in_=bass.AP(tensor=b1_d.ap().tensor, offset=0, ap=[[1, 128], [128, 16]]))
            b2r = glob.tile([1, 512], bf16, tag="b2r")
            nc.sync.dma_start(out=b2r, in_=b2_d.ap().unsqueeze(0))

        # persistent cross-phase big tensors (stacked so releases are LIFO)
        ffn_pool = tc.alloc_tile_pool(name="ffn_pool", bufs=1)
        w14 = ffn_pool.tile([128, 4, MLP], bf16, tag="w14")
        w2t = ffn_pool.tile([128, 16, C], bf16, tag="w2t")
        ytin = ffn_pool.tile([128, 4, N1], bf16, tag="ytin")   # [p, cc, tok]: y^T
        ytk = ffn_pool.tile([128, 32, 512], bf16, tag="ytk")   # [p, i, c]: y rows (then z2 in-place)
        qn_pool = tc.alloc_tile_pool(name="qn_pool", bufs=1)
        qn_sb = qn_pool.tile([128, 32, 512], bf16, tag="qn")  # qn_sb[p,i,:] = q[i*128+p,:]
        qt_pool = tc.alloc_tile_pool(name="qt_pool", bufs=1)
        qt_res = [qt_pool.tile([128, N1], bf16, tag=f"qtr{co}", name=f"qt_res{co}") for co in range(4)]
        # attention-output bounce buffer: rows d*64+m for d in 0..63, den at 4096+m
        xperm2 = nc.dram_tensor("xperm2", [65 * 64, 512], bf16)

        # ================= stage A: q^T (+ qnat via XBAR transpose) =================
        with tc.tile_pool(name="ab_sb", bufs=1) as ab:
            wq4 = ab.tile([128, 4, C], bf16, tag="wq4")
            wqr = Wq.ap().rearrange("(cc p) n -> p cc n", p=128)
            x1r = x1T.ap().rearrange("(cc p) n -> p cc n", p=128)
            # first compute tile's operands arrive first: wq cc=0 and x1 (nq0,cc0)
            nc.sync.dma_start(out=wq4[:, 0:1, :], in_=wqr[:, 0:1, :])
            x1t_first = ab.tile([128, 4, 512], bf16, tag="x1t4", bufs=2)
            nc.sync.dma_start(out=x1t_first[:, 0:1, :], in_=x1r[:, 0:1, 0:512])
            nc.sync.dma_start(out=wq4[:, 1:4, :], in_=wqr[:, 1:4, :])
            nc.sync.dma_start(out=x1t_first[:, 1:4, :], in_=x1r[:, 1:4, 0:512])
            aps_pool = tc.alloc_tile_pool(name="a_ps", bufs=1, space="PSUM")
            for nq in range(8):
                if nq == 0:
                    x1t4 = x1t_first
                else:
                    x1t4 = ab.tile([128, 4, 512], bf16, tag="x1t4", bufs=2)
                    nc.sync.dma_start(out=x1t4, in_=x1r[:, :, nq * 512:(nq + 1) * 512])
                if use_pos:
                    pqt_t = ab.tile([128, 4, 512], bf16, tag="pqt")
                    nc.sync.dma_start(out=pqt_t, in_=pqT_d.ap().rearrange("(a p) n -> p a n", p=128)[:, :, nq * 512:(nq + 1) * 512])
                for co in range(4):
                    ps = aps_pool.tile([128, 512], f32, tag="qt_ps", bufs=2)
                    for cc in range(4):
                        nc.tensor.matmul(ps, wq4[:, cc, co * 128:(co + 1) * 128], x1t4[:, cc, :],
                                         start=(cc == 0), stop=(cc == 3))
                    dst = qt_res[co][:, nq * 512:(nq + 1) * 512]
                    if use_bias:
                        nc.scalar.activation(dst, ps, AFT.Identity, bias=bq_sb[:, co:co + 1])
                        if use_pos:
                            nc.vector.tensor_add(dst, dst, pqt_t[:, co, :])
                    elif use_pos:
                        nc.vector.tensor_add(dst, ps, pqt_t[:, co, :])
                    else:
                        nc.scalar.activation(dst, ps, AFT.Identity)
            aps_pool.release()
            # qnat: qn_sb[p, i, co*128+c2] = qt_res[co][c2, i*128+p] via DMA XBAR transpose
            for co in range(4):
                nc.sync.dma_start(out=qn_sb[:, :, co * 128:(co + 1) * 128], in_=qt_res[co], transpose=True)

            # ================= stage B: k, v, kp, vpT =================
            bps_pool = tc.alloc_tile_pool(name="b_ps", bufs=1, space="PSUM")
            wkv4 = ab.tile([128, 4, 2 * C], bf16, tag="wkv4")
            nc.sync.dma_start(out=wkv4, in_=Wkv.ap().rearrange("(cc p) n -> p cc n", p=128))
            kp_ps = [bps_pool.tile([128, 256], f32, tag=f"kp_ps{j}", name=f"kp_ps{j}") for j in range(4)]
            vp_ps = [bps_pool.tile([128, 512], f32, tag=f"vp_ps{pc}", name=f"vp_ps{pc}") for pc in range(2)]
            x2t4 = wp4 = None
            for n2c in range(32):
                blk, sl = n2c // 4, n2c % 4
                if sl == 0:
                    x2t4 = ab.tile([128, 4, 512], bf16, tag="x2t4", bufs=2)
                    nc.sync.dma_start(out=x2t4, in_=x2T.ap().rearrange("(cc p) n -> p cc n", p=128)[:, :, blk * 512:(blk + 1) * 512])
                    wp4 = ab.tile([128, 4, 256], bf16, tag="wp4", bufs=2)
                    nc.sync.dma_start(out=wp4, in_=Wp.ap().rearrange("(a p) n -> p a n", p=128)[:, blk * 4:blk * 4 + 4, :])
                wp_in = wp4[:, sl, :]
                kps = bps_pool.tile([128, 512], f32, tag="k_ps", bufs=1)
                vps = bps_pool.tile([128, 512], f32, tag="v_ps", bufs=1)
                for cc in range(4):
                    nc.tensor.matmul(kps, x2t4[:, cc, sl * 128:(sl + 1) * 128], wkv4[:, cc, 0:512],
                                     start=(cc == 0), stop=(cc == 3 and not use_bias))
                if use_bias:
                    nc.tensor.matmul(kps, ones1b, bkvr[:, 0:512], start=False, stop=True, skip_group_check=True)
                for cc in range(4):
                    nc.tensor.matmul(vps, x2t4[:, cc, sl * 128:(sl + 1) * 128], wkv4[:, cc, 512:1024],
                                     start=(cc == 0), stop=(cc == 3 and not use_bias))
                if use_bias:
                    nc.tensor.matmul(vps, ones1b, bkvr[:, 512:1024], start=False, stop=True, skip_group_check=True)
                k_sb = ab.tile([128, 512], bf16, tag="k_sb", bufs=3)
                v_sb = ab.tile([128, 512], bf16, tag="v_sb", bufs=3)
                nc.scalar.activation(k_sb, kps, AFT.Identity)
                if use_pos:
                    pk_t = ab.tile([128, 512], bf16, tag="pkn", bufs=2)
                    nc.sync.dma_start(out=pk_t, in_=pkN_d.ap()[n2c * 128:(n2c + 1) * 128, :])
                    nc.vector.tensor_add(k_sb, k_sb, pk_t)
                nc.vector.tensor_copy(v_sb, vps)
                for hp in range(4):
                    nc.tensor.matmul(kp_ps[hp], k_sb[:, hp * 128:(hp + 1) * 128], wp_in,
                                     start=(n2c == 0), stop=(n2c == 31 and not use_bias))
                for pc in range(2):
                    nc.tensor.matmul(vp_ps[pc], wp_in[:, pc * 128:(pc + 1) * 128], v_sb,
                                     start=(n2c == 0), stop=(n2c == 31))
            if use_bias:
                for hp in range(4):
                    nc.tensor.matmul(kp_ps[hp], ones1b, bp2r[:, 0:256], start=False, stop=True, skip_group_check=True)
            for hp in range(4):
                nc.vector.tensor_copy(kp_sb[hp // 2][:, (hp % 2) * 256:(hp % 2) * 256 + 256], kp_ps[hp])
            for pc in range(2):
                vv = vpe[pc]
                if use_bias:
                    nc.scalar.activation(vv[:, :, 0:64], vp_ps[pc].rearrange("p (h e) -> p h e", e=64),
                                         AFT.Identity, bias=bp_sb[:, pc:pc + 1])
                else:
                    nc.scalar.activation(vv[:, :, 0:64], vp_ps[pc].rearrange("p (h e) -> p h e", e=64),
                                         AFT.Identity)
                of = glob.tile([128, 8], bf16, tag="onesf")
                nc.vector.memset(of, 1.0)
                nc.vector.tensor_copy(vv[:, :, 64:65], of.unsqueeze(2))
            bps_pool.release()

        # FFN weights load during the (activation-bound) attention phase
        nc.sync.dma_start(out=w14, in_=W1.ap().rearrange("(cc p) n -> p cc n", p=128))
        nc.sync.dma_start(out=w2t, in_=W2.ap().rearrange("(m p) n -> p m n", p=128))

        # ================= stage C: attention =================
        with tc.tile_pool(name="c_sb", bufs=1) as cp, \
             tc.tile_pool(name="c_ps", bufs=1, space="PSUM") as cps:
            for nq in range(8):
                xsm = cp.tile([65, 8, 512], bf16, tag="xsm", bufs=3)
                for t in range(4):
                    for hh in range(2):
                        h = 2 * t + hh
                        aps = cps.tile([128, 2, 512], f32, tag="attn_ps", bufs=3)
                        for pc in range(2):
                            nc.tensor.matmul(aps[:, pc, :],
                                             kp_sb[t // 2][hh * 64:(hh + 1) * 64, (t % 2) * 256 + pc * 128:(t % 2) * 256 + (pc + 1) * 128],
                                             qt_res[t][hh * 64:(hh + 1) * 64, nq * 512:(nq + 1) * 512],
                                             start=True, stop=True)
                        ebuf = cp.tile([128, 2, 512], bf16, tag="ebuf", bufs=6)
                        nc.scalar.activation(ebuf, aps, AFT.Exp, scale=float(temp_vals[h]))
                        xps = cps.tile([65, 512], f32, tag="x_ps", bufs=2)
                        nc.tensor.matmul(xps, vpe[0][:, h, :], ebuf[:, 0, :], start=True, stop=False)
                        nc.tensor.matmul(xps, vpe[1][:, h, :], ebuf[:, 1, :], start=False, stop=True)
                        nc.vector.tensor_copy(xsm[:, h, :], xps)
                nc.sync.dma_start(
                    out=bass.AP(tensor=xperm2.ap().tensor, offset=nq * 512,
                                ap=[[64 * 512, 65], [8 * 512, 8], [1, 512]]),
                    in_=xsm)
        qt_pool.release()

        # ================= stage D: permute + add&norm + y^T =================
        with tc.tile_pool(name="d_sb", bufs=1) as dp:
            denb = dp.tile([128, 512], bf16, tag="denb")
            for dl in range(2):
                nc.sync.dma_start(out=denb[dl * 64:(dl + 1) * 64, :], in_=xperm2.ap()[4096:4160, :])
            with nc.allow_low_precision(reason="softmax denominators are O(100+); bf16 reciprocal is plenty for 2e-2 tol"):
                nc.vector.reciprocal(drec, denb)
            for g in range(4):
                zts = []
                xt8 = dp.tile([128, 8, 512], bf16, tag="xt8", bufs=2)
                nc.sync.dma_start(out=xt8, in_=bass.AP(tensor=xperm2.ap().tensor, offset=g * 8 * 128 * 512,
                                                       ap=[[512, 128], [128 * 512, 8], [1, 512]]))
                for i in range(g * 8, g * 8 + 8):
                    zt = dp.tile([128, 512], bf16, tag=f"zt{i % 8}", name=f"zt{i % 8}", bufs=2)
                    nc.vector.tensor_mul(zt, xt8[:, i % 8, :], drec)
                    nc.vector.tensor_add(zt, zt, qn_sb[:, i, :])
                    sq = dp.tile([128, 512], bf16, tag="sq", bufs=3)
                    nc.scalar.activation(sq, zt, AFT.Square, accum_out=ss1[:, i:i + 1])
                    zts.append(zt)
                dve_rsqrt(rn1[:, g * 8:g * 8 + 8], ss1[:, g * 8:g * 8 + 8], dp, "d", 8)
                for i in range(g * 8, g * 8 + 8):
                    nc.vector.tensor_scalar_mul(ytk[:, i, :], zts[i - g * 8], rn1[:, i:i + 1])
                    nc.sync.dma_start(out=ytin[:, :, i * 128:(i + 1) * 128], in_=ytk[:, i, :], transpose=True)
        qn_pool.release()

        # ================= stage E: FFN + final norm =================
        with tc.tile_pool(name="e_sb", bufs=1) as ep, \
             tc.tile_pool(name="e_ps", bufs=1, space="PSUM") as eps:
            for nq in range(8):
                h1t = []
                for m in range(16):
                    ps = eps.tile([128, 512], f32, tag="h1_ps", bufs=4)
                    for cc in range(4):
                        nc.tensor.matmul(ps, w14[:, cc, m * 128:(m + 1) * 128],
                                         ytin[:, cc, nq * 512:(nq + 1) * 512],
                                         start=(cc == 0), stop=(cc == 3))
                    ht = ep.tile([128, 512], bf16, tag=f"h1t{m}", name=f"h1t{m}", bufs=2)
                    if use_bias:
                        nc.scalar.activation(ht, ps, AFT.Gelu, bias=b1_sb[:, m:m + 1])
                    else:
                        nc.scalar.activation(ht, ps, AFT.Gelu)
                    h1t.append(ht)
                for sub in range(4):
                    i = nq * 4 + sub
                    ps = eps.tile([128, 512], f32, tag="h2_ps", bufs=2)
                    for m in range(16):
                        nc.tensor.matmul(ps, h1t[m][:, sub * 128:(sub + 1) * 128], w2t[:, m, :],
                                         start=(m == 0), stop=(m == 15 and not use_bias))
                    if use_bias:
                        nc.tensor.matmul(ps, ones1b, b2r, start=False, stop=True, skip_group_check=True)
                    z2 = ytk[:, i, :]
                    nc.vector.tensor_add(z2, ps, ytk[:, i, :])
                    sq2 = ep.tile([128, 512], bf16, tag="sq2", bufs=2)
                    nc.scalar.activation(sq2, z2, AFT.Square, accum_out=ss2[:, i:i + 1])
                # flush final norms for this nq block (DVE-only rsqrt; no
                # act-table switch) so the output tail overlaps the FFN
                g4 = nq * 4
                dve_rsqrt(rn2[:, g4:g4 + 4], ss2[:, g4:g4 + 4], ep, "e", 4)
                ot4 = ep.tile([128, 4, 512], bf16, tag="ot4", bufs=2)
                for i in range(g4, g4 + 4):
                    nc.vector.tensor_scalar_mul(ot4[:, i - g4, :], ytk[:, i, :], rn2[:, i:i + 1])
                nc.sync.dma_start(out=bass.AP(tensor=out_d.ap().tensor, offset=nq * 4 * 128 * 512,
                                              ap=[[512, 128], [128 * 512, 4], [1, 512]]),
                                  in_=ot4)
        ffn_pool.release()
        glob.release()
    nc.compile()
    return nc


def kernel(**inputs):
    from concourse.bass_utils import run_bass_kernel_spmd

    bf = ml_dtypes.bfloat16
    x1 = np.asarray(inputs["x1"], np.float32)
    x2 = np.asarray(inputs["x2"], np.float32)
    temp = np.asarray(inputs["temperature"], np.float32).reshape(H)
    biases = [np.asarray(inputs[k], np.float32) for k in ("bq", "bkv", "bp", "b1", "b2")]
    use_bias = any(np.any(b) for b in biases)
    pos_q = np.asarray(inputs["pos_q"], np.float32).reshape(N1, C)
    pos_k = np.asarray(inputs["pos_k"], np.float32).reshape(N2, C)
    use_pos = bool(np.any(pos_q) or np.any(pos_k))

    key = (tuple(np.round(temp, 7).tolist()), use_bias, use_pos)
    if key not in _CACHE:
        _CACHE[key] = _build(temp, use_bias, use_pos)
    nc = _CACHE[key]

    shared = {
        "Wq": np.ascontiguousarray(inputs["Wq"]).astype(bf),
        "Wkv": np.ascontiguousarray(inputs["Wkv"]).astype(bf),
        "Wp": np.ascontiguousarray(inputs["Wp"]).astype(bf),
        "W1": np.ascontiguousarray(inputs["W1"]).astype(bf),
        "W2": np.ascontiguousarray(inputs["W2"]).astype(bf),
    }
    if use_bias:
        shared.update(bq=biases[0], bkv2=biases[1].astype(bf), bp=biases[2],
                      bp2=biases[2].astype(bf), b1=biases[3], b2=biases[4].astype(bf))
    if use_pos:
        shared.update(pqT=np.ascontiguousarray(pos_q.T).astype(bf), pkN=pos_k.astype(bf))
    in_maps = []
    for b in range(B):
        m = dict(shared)
        m["x1T"] = np.ascontiguousarray(x1[b].T).astype(bf)
        m["x2T"] = np.ascontiguousarray(x2[b].T).astype(bf)
        in_maps.append(m)
    res = run_bass_kernel_spmd(nc, in_maps, core_ids=list(range(B)))
    return np.stack([np.asarray(res.results[b]["out"]).astype(np.float32) for b in range(B)])
